# revision 9
# baseline (speedup 1.0000x reference)
"""Distributed 2-layer GCN (PyG GCNConv-style) on 8 Trainium2 NeuronCores.

Strategy (v2 — optimized from the 465us baseline):
  - Nodes are sharded 2500/core, but first PERMUTED: a snake-deal over
    in-degree-sorted nodes balances edge counts per core (+-3 edges) and
    per 128-node dst block (max 1252 vs 1494 unbalanced), cutting the
    uniform edge-tile count KT from 12 to 10.
  - Self loops are removed from the edge slots (another KT reduction and
    ~5 MB less gather traffic); their contribution dis^2*x / dis^2*hw is
    added with DVE element-wise ops instead (host ships a dis^2-scaled
    feature-major copy of x, and dis^2 per dst).
  - Edges are partitioned by destination; per (core, dst-block) the
    incoming edges are packed into KT tiles of 128 slots. Host-built
    selection matrices S [slot, dst] turn scatter-add into TensorE
    matmuls (aggregation in INPUT space for layer 1, width F=512; in
    OUTPUT space for layer 2, width 128).
  - Layer-1 dense matmuls are batched over superblocks of 4 dst blocks:
    free dim 512 instead of 128 (~2.4x PE throughput there).
  - The hw = h@W2 table is AllGathered in NSB=5 per-superblock chunks so
    the collective overlaps phase-A compute; only the last chunk's tail
    is exposed before phase B.
  - bf16 everywhere on device (3e-3 rel err vs fp32 reference); fp32
    PSUM accumulation.

kernel(**inputs) takes the FULL unsharded inputs and returns the FULL
[20000, 128] float32 output.
"""

import math

import numpy as np
import ml_dtypes

import concourse.bass as bass
import concourse.mybir as mybir
import concourse.tile as tile
from concourse import bacc
from concourse.bass_utils import run_bass_kernel_spmd

# ----------------------------------------------------------------------------
# configuration
# ----------------------------------------------------------------------------
C = 8  # cores

COMPUTE = "bf16"
FP8_GATHER = True  # fp8 wire for the x-gather table and the hw AllGather

_DT = {
    "bf16": (mybir.dt.bfloat16, ml_dtypes.bfloat16),
    "f32": (mybir.dt.float32, np.float32),
}

_prog_cache: dict = {}


def _cfg_from_shapes(x, w1, w2):
    n, f = x.shape
    h = w1.shape[1]
    out = w2.shape[1]
    assert n % C == 0, n
    nl = n // C                      # real nodes per core
    nlp = ((nl + 127) // 128) * 128  # padded nodes per core
    b = nlp // 128                   # dst blocks per core
    sbb = min(4, b)                  # blocks per superblock
    while b % sbb:
        sbb -= 1
    assert f % 128 == 0 and h % 128 == 0 and out % 128 == 0
    assert out <= 512
    return dict(N=n, F=f, H=h, OUT=out, NL=nl, NLP=nlp, B=b, NP=C * nlp,
                FK=f // 128, HC=h // 128, OC=out // 128,
                SBB=sbb, NSB=b // sbb, SBW=sbb * 128)


# ----------------------------------------------------------------------------
# host-side preprocessing: balanced partition + norm coeffs + layouts
# ----------------------------------------------------------------------------
def _snake(n_items, n_bins):
    """Deal n_items (in priority order) across n_bins, snaking."""
    assert n_items % n_bins == 0
    rounds = n_items // n_bins
    pos = np.arange(n_items).reshape(rounds, n_bins)
    pos[1::2] = pos[1::2, ::-1]
    bins = np.empty(n_items, np.int64)
    bins[pos.reshape(-1)] = np.tile(np.arange(n_bins), rounds)
    rnd = np.empty(n_items, np.int64)
    rnd[pos.reshape(-1)] = np.repeat(np.arange(rounds), n_bins)
    return bins, rnd


def _preprocess(x, edge_index, edge_weight, w1, b1, w2, b2, rw1, rb1, rw2, rb2,
                cfg, np_cdt):
    N, F, H, OUT = cfg["N"], cfg["F"], cfg["H"], cfg["OUT"]
    NL, NLP, B, NP = cfg["NL"], cfg["NLP"], cfg["B"], cfg["NP"]
    HC, FK, SBW = cfg["HC"], cfg["FK"], cfg["SBW"]

    row = np.asarray(edge_index[0], dtype=np.int64)
    col = np.asarray(edge_index[1], dtype=np.int64)
    ew = np.asarray(edge_weight, dtype=np.float32)

    # symmetric normalization, exactly like the reference (self loop wt 1)
    deg = np.bincount(col, weights=ew.astype(np.float64), minlength=N) + 1.0
    deg = deg.astype(np.float32)
    dis = np.where(deg > 0, 1.0 / np.sqrt(np.where(deg > 0, deg, 1.0)), 0.0)
    dis = dis.astype(np.float32)

    # ---- balanced node partition (snake-deal by in-degree) ----
    indeg = np.bincount(col, minlength=N)
    order = np.argsort(-indeg, kind="stable")
    core_bins, _ = _snake(N, C)
    core_of = np.empty(N, np.int64)
    core_of[order] = core_bins
    local_of = np.empty(N, np.int64)
    for c in range(C):
        nodes = order[core_bins == c]          # degree-sorted nodes of core c
        blk, rnd = _snake(NL, B)               # block + rank-within-block
        local_of[nodes] = blk * 128 + rnd
    assert local_of.max() < NLP

    # ---- edge packing (by destination core/block), no self loops ----
    score = core_of[row]
    sl = local_of[row]
    srcA = score * NLP + sl                    # row in x_table
    sb_s = sl // SBW                           # row in chunked hw table
    srcB = sb_s * (C * SBW) + score * SBW + (sl % SBW)

    dcore = core_of[col]
    dl = local_of[col]
    dblk = dl // 128
    dloc = dl % 128
    norms = dis[row] * ew * dis[col]

    key = dcore * B + dblk
    order_e = np.argsort(key, kind="stable")
    key_s = key[order_e]
    counts = np.bincount(key_s, minlength=C * B)
    starts = np.zeros(C * B, dtype=np.int64)
    np.cumsum(counts[:-1], out=starts[1:])
    pos = np.arange(key_s.size, dtype=np.int64) - starts[key_s]

    KT = max(1, int(math.ceil(counts.max() / 128)))

    srcA_s = srcA[order_e].astype(np.int64)
    srcB_s = srcB[order_e].astype(np.int64)
    dcore_s = dcore[order_e]
    dblk_s = dblk[order_e]
    dloc_s = dloc[order_e]
    norm_s = norms[order_e]
    kt_s = pos // 128
    p_s = pos % 128

    # int16 indices for dma_gather: slot i of block b -> [i%16, b*KT*8+i//16],
    # replicated across the 8 groups of 16 partitions
    assert NP < 2 ** 15
    slot = kt_s * 128 + p_s

    def _mk_idx(vals):
        i16 = np.zeros((C, 16, B * KT * 8), dtype=np.int16)
        i16[dcore_s, slot % 16, dblk_s * (KT * 8) + slot // 16] = \
            vals.astype(np.int16)
        return np.tile(i16, (1, 8, 1))

    idxa_all = _mk_idx(srcA_s)
    idxb_all = _mk_idx(srcB_s)

    # S_all[c, p, b, kt*128+d] = norm  (partition-major, contiguous per core)
    S_all = np.zeros((C, 128, B, KT * 128), dtype=np.float32)
    S_all[dcore_s, p_s, dblk_s, kt_s * 128 + dloc_s] = norm_s
    S_all = S_all.astype(np_cdt)

    # ---- node tables (permuted) ----
    x = np.asarray(x, dtype=np.float32)
    x_table = np.zeros((NP, F), dtype=np.float32)
    x_table[core_of * NLP + local_of] = x

    dis2_table = np.zeros(NP, dtype=np.float32)
    dis2_table[core_of * NLP + local_of] = dis * dis

    xd2_table = x_table * dis2_table[:, None]

    fp8 = FP8_GATHER and np_cdt == ml_dtypes.bfloat16
    np_gdt = mybir.dt.np(mybir.dt.float8e4) if fp8 else np_cdt
    x_table_c = x_table.astype(np_gdt)
    # feature-major per core: xT[p, k, n] = x_core[n, k*128+p]
    xT_all = np.ascontiguousarray(
        x_table.astype(np_cdt).reshape(C, NLP, FK, 128).transpose(0, 3, 2, 1))
    xd2T_all = np.ascontiguousarray(
        xd2_table.astype(np_cdt).reshape(C, NLP, FK, 128).transpose(0, 3, 2, 1))
    dis2_in_all = np.ascontiguousarray(
        dis2_table.reshape(C, B, 128).transpose(0, 2, 1))  # [C, 128, B]

    w1 = np.asarray(w1, np.float32)
    rw1 = np.asarray(rw1, np.float32)
    w2 = np.asarray(w2, np.float32)
    rw2 = np.asarray(rw2, np.float32)
    b1c = (np.asarray(b1, np.float32) + np.asarray(rb1, np.float32))
    b2c = (np.asarray(b2, np.float32) + np.asarray(rb2, np.float32))

    # [128, FK, H] : w1_in[p, k, j] = w1[k*128+p, j]
    w1_in = np.ascontiguousarray(
        w1.reshape(FK, 128, H).transpose(1, 0, 2)).astype(np_cdt)
    rw1_in = np.ascontiguousarray(
        rw1.reshape(FK, 128, H).transpose(1, 0, 2)).astype(np_cdt)
    w2_in = np.ascontiguousarray(
        w2.reshape(HC, 128, OUT).transpose(1, 0, 2)).astype(np_cdt)
    rw2b_in = np.zeros((128, HC + 1, OUT), dtype=np.float32)
    rw2b_in[:, :HC] = rw2.reshape(HC, 128, OUT).transpose(1, 0, 2)
    rw2b_in[0, HC, :] = b2c
    rw2b_in = rw2b_in.astype(np_cdt)

    bias1_in = np.ascontiguousarray(b1c.reshape(HC, 128).T).astype(np.float32)

    in_maps = []
    for c in range(C):
        in_maps.append({
            "x_table": x_table_c,
            "idxa_in": np.ascontiguousarray(idxa_all[c]),
            "idxb_in": np.ascontiguousarray(idxb_all[c]),
            "s_in": np.ascontiguousarray(S_all[c]),
            "xt_in": np.ascontiguousarray(xT_all[c]),
            "xd2t_in": np.ascontiguousarray(xd2T_all[c]),
            "dis2_in": dis2_in_all[c],
            "w1_in": w1_in,
            "rw1_in": rw1_in,
            "w2_in": w2_in,
            "rw2b_in": rw2b_in,
            "bias1_in": bias1_in,
        })
    meta = {"core_of": core_of, "local_of": local_of}
    return in_maps, KT, meta


def _assemble(outs_per_core, cfg, meta):
    """outs_per_core[c] is the [NLP, OUT] 'out' tensor of core c; undo the
    node permutation and return the [N, OUT] full output."""
    N, NLP, OUT = cfg["N"], cfg["NLP"], cfg["OUT"]
    stacked = np.stack([np.asarray(o) for o in outs_per_core])  # [C, NLP, OUT]
    full = stacked.reshape(C * NLP, OUT)
    idx = meta["core_of"] * NLP + meta["local_of"]
    return np.ascontiguousarray(full[idx].astype(np.float32))


# ----------------------------------------------------------------------------
# device program
# ----------------------------------------------------------------------------
def _build(cfg, KT, cdt, reps=1, no_collective=False, phases="ab"):
    F, H, OUT = cfg["F"], cfg["H"], cfg["OUT"]
    NLP, B, NP = cfg["NLP"], cfg["B"], cfg["NP"]
    FK, HC = cfg["FK"], cfg["HC"]
    SBB, NSB, SBW = cfg["SBB"], cfg["NSB"], cfg["SBW"]
    f32 = mybir.dt.float32
    fp8 = FP8_GATHER and cdt == mybir.dt.bfloat16
    gdt = mybir.dt.float8e4 if fp8 else cdt

    # 32 KB SWDGE scratch = 2048-descriptor ring: a half-block gather
    # (ceil(KT/2)*128 = ~640 descriptors in one instruction) must fit.
    nc = bacc.Bacc("TRN2", target_bir_lowering=False, debug=False,
                   enable_asserts=False, num_devices=C,
                   dynamic_dma_scratch_size=32768, num_swdge_queues=2)

    x_table = nc.dram_tensor("x_table", [NP, F], gdt, kind="ExternalInput")
    idxa_in = nc.dram_tensor("idxa_in", [128, B * KT * 8], mybir.dt.int16,
                             kind="ExternalInput")
    idxb_in = nc.dram_tensor("idxb_in", [128, B * KT * 8], mybir.dt.int16,
                             kind="ExternalInput")
    s_in = nc.dram_tensor("s_in", [128, B, KT * 128], cdt,
                          kind="ExternalInput")
    xt_in = nc.dram_tensor("xt_in", [128, FK, NLP], cdt, kind="ExternalInput")
    xd2t_in = nc.dram_tensor("xd2t_in", [128, FK, NLP], cdt,
                             kind="ExternalInput")
    dis2_in = nc.dram_tensor("dis2_in", [128, B], f32, kind="ExternalInput")
    w1_in = nc.dram_tensor("w1_in", [128, FK, H], cdt, kind="ExternalInput")
    rw1_in = nc.dram_tensor("rw1_in", [128, FK, H], cdt, kind="ExternalInput")
    w2_in = nc.dram_tensor("w2_in", [128, HC, OUT], cdt, kind="ExternalInput")
    rw2b_in = nc.dram_tensor("rw2b_in", [128, HC + 1, OUT], cdt,
                             kind="ExternalInput")
    bias1_in = nc.dram_tensor("bias1_in", [128, HC], f32,
                              kind="ExternalInput")
    out_d = nc.dram_tensor("out", [NLP, OUT], f32, kind="ExternalOutput")

    def _gather(out_tile, table, idx_sb, b, elem, qn):
        h0 = KT // 2
        ranges = [(0, h0), (h0, KT)] if h0 > 0 else [(0, KT)]
        for lo, hi in ranges:
            nc.gpsimd.dma_gather(
                out_ap=out_tile[:, lo:hi], in_ap=table[:],
                idxs_ap=idx_sb[:, b * KT * 8 + lo * 8:
                               b * KT * 8 + hi * 8],
                num_idxs=(hi - lo) * 128, num_idxs_reg=(hi - lo) * 128,
                elem_size=elem, single_packet=False,
                queue_num=qn)

    with tile.TileContext(nc) as tc:
        with (
            tc.tile_pool(name="dram", bufs=1, space="DRAM") as dram,
            tc.tile_pool(name="const", bufs=1) as const,
        ):
            # resident constants
            w1_sb = const.tile([128, FK, H], cdt)
            nc.sync.dma_start(out=w1_sb[:], in_=w1_in[:])
            rw1_sb = const.tile([128, FK, H], cdt)
            nc.sync.dma_start(out=rw1_sb[:], in_=rw1_in[:])
            w2_sb = const.tile([128, HC, OUT], cdt)
            nc.sync.dma_start(out=w2_sb[:], in_=w2_in[:])
            rw2b_sb = const.tile([128, HC + 1, OUT], cdt)
            nc.sync.dma_start(out=rw2b_sb[:], in_=rw2b_in[:])
            bias1_sb = const.tile([128, HC], f32)
            nc.sync.dma_start(out=bias1_sb[:], in_=bias1_in[:])
            dis2_sb = const.tile([128, B], f32)
            nc.sync.dma_start(out=dis2_sb[:], in_=dis2_in[:])
            idxa_sb = const.tile([128, B * KT * 8], mybir.dt.int16)
            nc.sync.dma_start(out=idxa_sb[:], in_=idxa_in[:])
            idxb_sb = const.tile([128, B * KT * 8], mybir.dt.int16)
            nc.sync.dma_start(out=idxb_sb[:], in_=idxb_in[:])
            ones_sb = const.tile([128, 128], cdt)
            nc.vector.memset(ones_sb[:], 0.0)
            nc.vector.memset(ones_sb[0:1, :], 1.0)

            # resident: S (norm selection matrices), hT (post-relu h,
            # feature-major), local hw rows (node-major)
            s_all_sb = const.tile([128, B, KT * 128], cdt)
            nc.sync.dma_start(out=s_all_sb[:], in_=s_in[:])
            hT_all = const.tile([128, HC, NLP], cdt)
            hwloc_sb = const.tile([128, B, OUT], cdt)

            for rep in range(reps):
                hw_loc = dram.tile([NLP, OUT], gdt, tag="hw_loc",
                                   name=f"hw_loc{rep}")
                # phase-B gather table (Local: many writers allowed). Each
                # superblock's AllGather lands in its own small Shared tensor
                # (single-writer rule), then is copied into the table.
                hw_full = dram.tile([NSB * C * SBW, OUT], cdt,
                                    tag="hw_full", name=f"hw_full{rep}")
                # ---------------- phase A: layer 1 + hw + chunked AG ------
                with (
                    tc.tile_pool(name=f"xg_pool{rep}", bufs=2) as xg_pool,
                    tc.tile_pool(name=f"xt_pool{rep}", bufs=1) as xt_pool,
                    tc.tile_pool(name=f"ax_pool{rep}", bufs=2) as ax_pool,
                    tc.tile_pool(name=f"ax_psum{rep}", bufs=2,
                                 space="PSUM") as ax_psum,
                    tc.tile_pool(name=f"h_psum{rep}", bufs=2,
                                 space="PSUM") as h_psum,
                    tc.tile_pool(name=f"hw_psum{rep}", bufs=2,
                                 space="PSUM") as hw_psum,
                    tc.tile_pool(name=f"osb8_pool{rep}", bufs=2) as osb8_pool,
                ):
                    for sb in range(NSB):
                        ss = slice(sb * SBW, (sb + 1) * SBW)
                        xt4 = xt_pool.tile([128, FK, SBW], cdt, tag="xt4")
                        nc.sync.dma_start(out=xt4[:], in_=xt_in[:, :, ss])
                        xd2t4 = xt_pool.tile([128, FK, SBW], cdt, tag="xd2t4")
                        nc.sync.dma_start(out=xd2t4[:], in_=xd2t_in[:, :, ss])
                        axT4 = ax_pool.tile([128, FK, SBW], cdt, tag="axT4")

                        for j in range(SBB):
                            b = sb * SBB + j
                            xg8 = xg_pool.tile([128, KT, F], gdt, tag="xg8")
                            _gather(xg8, x_table, idxa_sb, b, F, b % 2)
                            if fp8:
                                xg = xg_pool.tile([128, KT, F], cdt, tag="xg")
                                h0 = max(KT // 2, 1)
                                nc.vector.tensor_copy(out=xg[:, 0:h0],
                                                      in_=xg8[:, 0:h0])
                                if h0 < KT:
                                    nc.vector.tensor_copy(out=xg[:, h0:],
                                                          in_=xg8[:, h0:])
                            else:
                                xg = xg8
                            psum_ax = ax_psum.tile([128, FK, 128], f32,
                                                   tag="psum_ax")
                            for fc in range(FK):
                                for kt in range(KT):
                                    nc.tensor.matmul(
                                        out=psum_ax[:, fc, :],
                                        lhsT=xg[:, kt, fc * 128:(fc + 1) * 128],
                                        rhs=s_all_sb[:, b,
                                                     kt * 128:(kt + 1) * 128],
                                        start=(kt == 0), stop=(kt == KT - 1))
                            # PSUM evac + self-loop add (dis^2 * x), cast bf16
                            nc.vector.tensor_tensor(
                                out=axT4[:, :, j * 128:(j + 1) * 128],
                                in0=psum_ax[:],
                                in1=xd2t4[:, :, j * 128:(j + 1) * 128],
                                op=mybir.AluOpType.add)

                        # dense: hT = relu(W1.T @ axT + RW1.T @ xT + b1c),
                        # free dim SBW=512
                        for hc in range(HC):
                            hs = slice(hc * 128, (hc + 1) * 128)
                            psum_h = h_psum.tile([128, SBW], f32, tag="psum_h")
                            for k in range(FK):
                                nc.tensor.matmul(
                                    out=psum_h[:], lhsT=w1_sb[:, k, hs],
                                    rhs=axT4[:, k, :],
                                    start=(k == 0), stop=False)
                            for k in range(FK):
                                nc.tensor.matmul(
                                    out=psum_h[:], lhsT=rw1_sb[:, k, hs],
                                    rhs=xt4[:, k, :],
                                    start=False, stop=(k == FK - 1))
                            nc.scalar.activation(
                                out=hT_all[:, hc, ss], in_=psum_h[:],
                                func=mybir.ActivationFunctionType.Relu,
                                bias=bias1_sb[:, hc:hc + 1], scale=1.0)

                        # hw = h @ W2 (node-major), per block
                        for j in range(SBB):
                            b = sb * SBB + j
                            bw = slice(b * 128, (b + 1) * 128)
                            psum_hw = hw_psum.tile([128, OUT], f32,
                                                   tag="psum_hw")
                            for hc in range(HC):
                                nc.tensor.matmul(
                                    out=psum_hw[:], lhsT=hT_all[:, hc, bw],
                                    rhs=w2_sb[:, hc, :],
                                    start=(hc == 0), stop=(hc == HC - 1))
                            nc.vector.tensor_copy(out=hwloc_sb[:, b, :],
                                                  in_=psum_hw[:])
                            if fp8:
                                hw8 = osb8_pool.tile([128, OUT], gdt,
                                                     tag="hw8")
                                nc.vector.tensor_copy(out=hw8[:],
                                                      in_=psum_hw[:])
                                nc.sync.dma_start(out=hw_loc[bw, :],
                                                  in_=hw8[:])
                            else:
                                nc.sync.dma_start(out=hw_loc[bw, :],
                                                  in_=hwloc_sb[:, b, :])

                        # all-gather this superblock's hw chunk
                        if no_collective:
                            nc.gpsimd.dma_start(
                                out=hw_full[sb * C * SBW:sb * C * SBW + SBW, :],
                                in_=hw_loc[ss, :])
                        else:
                            hw_ag = dram.tile([C * SBW, OUT], gdt,
                                              addr_space="Shared", tag="hw_ag",
                                              name=f"hw_ag{rep}_{sb}")
                            nc.gpsimd.collective_compute(
                                "AllGather",
                                mybir.AluOpType.bypass,
                                replica_groups=[list(range(C))],
                                ins=[hw_loc[ss, :].opt()],
                                outs=[hw_ag[:].opt()],
                            )
                            if fp8:
                                nc.gpsimd.dma_start(
                                    out=hw_full[sb * C * SBW:
                                                (sb + 1) * C * SBW, :],
                                    in_=hw_ag[:])
                            else:
                                nc.sync.dma_start(
                                    out=hw_full[sb * C * SBW:
                                                (sb + 1) * C * SBW, :],
                                    in_=hw_ag[:])

                # ---------------- phase B: layer 2 ----------------
                if phases == "a":
                    continue
                with (
                    tc.tile_pool(name=f"hwg_pool{rep}", bufs=4) as hwg_pool,
                    tc.tile_pool(name=f"res_pool{rep}", bufs=1) as res_pool,
                    tc.tile_pool(name=f"osb_pool{rep}", bufs=3) as osb_pool,
                    tc.tile_pool(name=f"r_psum{rep}", bufs=2,
                                 space="PSUM") as r_psum,
                    tc.tile_pool(name=f"o_psum{rep}", bufs=3,
                                 space="PSUM") as o_psum,
                ):
                    # residual h@RW2 + b2c + dis^2*hw: depends only on local
                    # data, so it fills the PE-idle window while the last
                    # AllGather chunks are in flight.
                    res_all = res_pool.tile([128, B, OUT], f32, tag="res_all")
                    for b in range(B):
                        bw = slice(b * 128, (b + 1) * 128)
                        psum_r = r_psum.tile([128, OUT], f32, tag="psum_r")
                        for k in range(HC):
                            nc.tensor.matmul(
                                out=psum_r[:], lhsT=hT_all[:, k, bw],
                                rhs=rw2b_sb[:, k, 0:OUT],
                                start=(k == 0), stop=False)
                        nc.tensor.matmul(
                            out=psum_r[:], lhsT=ones_sb[:],
                            rhs=rw2b_sb[:, HC, 0:OUT],
                            start=False, stop=True)
                        # + self loop term dis^2 * hw_local
                        tmp = osb_pool.tile([128, OUT], f32, tag="tmp")
                        nc.vector.tensor_scalar(
                            out=tmp[:], in0=hwloc_sb[:, b, :],
                            scalar1=dis2_sb[:, b:b + 1], scalar2=None,
                            op0=mybir.AluOpType.mult)
                        nc.vector.tensor_tensor(
                            out=res_all[:, b, :], in0=psum_r[:], in1=tmp[:],
                            op=mybir.AluOpType.add)

                    for b in range(B):
                        hwg = hwg_pool.tile([128, KT, OUT], cdt, tag="hwg")
                        _gather(hwg, hw_full, idxb_sb, b, OUT, b % 2)

                        psum_o = o_psum.tile([128, OUT], f32, tag="psum_o")
                        for kt in range(KT):
                            nc.tensor.matmul(
                                out=psum_o[:],
                                lhsT=s_all_sb[:, b, kt * 128:(kt + 1) * 128],
                                rhs=hwg[:, kt, :],
                                start=(kt == 0), stop=(kt == KT - 1))
                        out_sb = osb_pool.tile([128, OUT], f32, tag="out_sb")
                        nc.vector.tensor_tensor(
                            out=out_sb[:], in0=psum_o[:],
                            in1=res_all[:, b, :],
                            op=mybir.AluOpType.add)
                        nc.sync.dma_start(out=out_d[b * 128:(b + 1) * 128, :],
                                          in_=out_sb[:])

    nc.compile()
    return nc


# ----------------------------------------------------------------------------
# entry points
# ----------------------------------------------------------------------------
def _run(inputs, trace=False, compute=None, trace_kwargs=None):
    compute = compute or COMPUTE
    cdt, np_cdt = _DT[compute]
    x = np.asarray(inputs["x"])
    cfg = _cfg_from_shapes(x, np.asarray(inputs["w1"]),
                           np.asarray(inputs["w2"]))
    in_maps, KT, meta = _preprocess(
        x, inputs["edge_index"], inputs["edge_weight"],
        inputs["w1"], inputs["b1"], inputs["w2"], inputs["b2"],
        inputs["rw1"], inputs["rb1"], inputs["rw2"], inputs["rb2"],
        cfg, np_cdt)

    key = (tuple(sorted(cfg.items())), KT, compute)
    nc = _prog_cache.get(key)
    if nc is None:
        nc = _build(cfg, KT, cdt)
        _prog_cache[key] = nc

    res = run_bass_kernel_spmd(
        nc, in_maps, core_ids=list(range(C)), trace=trace,
        **(trace_kwargs or {}))

    out = _assemble([res.results[c]["out"] for c in range(C)], cfg, meta)
    return out, res


def kernel(**inputs) -> np.ndarray:
    out, _ = _run(inputs, trace=False)
    return out


# revision 19
# speedup vs baseline: 1.9583x; 1.9583x over previous
"""Distributed 2-layer GCN (PyG GCNConv-style) on 8 Trainium2 NeuronCores.

Strategy (v2 — optimized from the 465us baseline):
  - Nodes are sharded 2500/core, but first PERMUTED: a snake-deal over
    in-degree-sorted nodes balances edge counts per core (+-3 edges) and
    per 128-node dst block (max 1252 vs 1494 unbalanced), cutting the
    uniform edge-tile count KT from 12 to 10.
  - Self loops are removed from the edge slots (another KT reduction and
    ~5 MB less gather traffic); their contribution dis^2*x / dis^2*hw is
    added with DVE element-wise ops instead (host ships a dis^2-scaled
    feature-major copy of x, and dis^2 per dst).
  - Edges are partitioned by destination; per (core, dst-block) the
    incoming edges are packed into KT tiles of 128 slots. Host-built
    selection matrices S [slot, dst] turn scatter-add into TensorE
    matmuls (aggregation in INPUT space for layer 1, width F=512; in
    OUTPUT space for layer 2, width 128).
  - Layer-1 dense matmuls are batched over superblocks of 4 dst blocks:
    free dim 512 instead of 128 (~2.4x PE throughput there).
  - The hw = h@W2 table is AllGathered in NSB=5 per-superblock chunks so
    the collective overlaps phase-A compute; only the last chunk's tail
    is exposed before phase B.
  - bf16 everywhere on device (3e-3 rel err vs fp32 reference); fp32
    PSUM accumulation.

kernel(**inputs) takes the FULL unsharded inputs and returns the FULL
[20000, 128] float32 output.
"""

import math

import numpy as np
import ml_dtypes

import concourse.bass as bass
import concourse.mybir as mybir
import concourse.tile as tile
from concourse import bacc
from concourse.bass_utils import run_bass_kernel_spmd

# ----------------------------------------------------------------------------
# configuration
# ----------------------------------------------------------------------------
C = 8  # cores

COMPUTE = "bf16"
FP8_GATHER = True  # fp8 wire for the x-gather table and the hw AllGather

_DT = {
    "bf16": (mybir.dt.bfloat16, ml_dtypes.bfloat16),
    "f32": (mybir.dt.float32, np.float32),
}

_prog_cache: dict = {}


def _cfg_from_shapes(x, w1, w2):
    n, f = x.shape
    h = w1.shape[1]
    out = w2.shape[1]
    assert n % C == 0, n
    nl = n // C                      # real nodes per core
    nlp = ((nl + 127) // 128) * 128  # padded nodes per core
    b = nlp // 128                   # dst blocks per core
    sbb = min(4, b)                  # blocks per superblock
    while b % sbb:
        sbb -= 1
    assert f % 128 == 0 and h % 128 == 0 and out % 128 == 0
    assert out <= 512
    return dict(N=n, F=f, H=h, OUT=out, NL=nl, NLP=nlp, B=b, NP=C * nlp,
                FK=f // 128, HC=h // 128, OC=out // 128,
                SBB=sbb, NSB=b // sbb, SBW=sbb * 128)


# ----------------------------------------------------------------------------
# host-side preprocessing: balanced partition + norm coeffs + layouts
# ----------------------------------------------------------------------------
def _snake(n_items, n_bins):
    """Deal n_items (in priority order) across n_bins, snaking."""
    assert n_items % n_bins == 0
    rounds = n_items // n_bins
    pos = np.arange(n_items).reshape(rounds, n_bins)
    pos[1::2] = pos[1::2, ::-1]
    bins = np.empty(n_items, np.int64)
    bins[pos.reshape(-1)] = np.tile(np.arange(n_bins), rounds)
    rnd = np.empty(n_items, np.int64)
    rnd[pos.reshape(-1)] = np.repeat(np.arange(rounds), n_bins)
    return bins, rnd


def _preprocess(x, edge_index, edge_weight, w1, b1, w2, b2, rw1, rb1, rw2, rb2,
                cfg, np_cdt):
    N, F, H, OUT = cfg["N"], cfg["F"], cfg["H"], cfg["OUT"]
    NL, NLP, B, NP = cfg["NL"], cfg["NLP"], cfg["B"], cfg["NP"]
    HC, FK, SBW = cfg["HC"], cfg["FK"], cfg["SBW"]

    row = np.asarray(edge_index[0], dtype=np.int64)
    col = np.asarray(edge_index[1], dtype=np.int64)
    ew = np.asarray(edge_weight, dtype=np.float32)

    # symmetric normalization, exactly like the reference (self loop wt 1)
    deg = np.bincount(col, weights=ew.astype(np.float64), minlength=N) + 1.0
    deg = deg.astype(np.float32)
    dis = np.where(deg > 0, 1.0 / np.sqrt(np.where(deg > 0, deg, 1.0)), 0.0)
    dis = dis.astype(np.float32)

    # ---- balanced node partition (snake-deal by in-degree) ----
    indeg = np.bincount(col, minlength=N)
    order = np.argsort(-indeg, kind="stable")
    core_bins, _ = _snake(N, C)
    core_of = np.empty(N, np.int64)
    core_of[order] = core_bins
    local_of = np.empty(N, np.int64)
    for c in range(C):
        nodes = order[core_bins == c]          # degree-sorted nodes of core c
        blk, rnd = _snake(NL, B)               # block + rank-within-block
        local_of[nodes] = blk * 128 + rnd
    assert local_of.max() < NLP

    # ---- edge packing (by destination core/block), no self loops ----
    score = core_of[row]
    sl = local_of[row]
    srcA = score * NLP + sl                    # row in x_table
    sb_s = sl // SBW                           # row in chunked hw table
    srcB = sb_s * (C * SBW) + score * SBW + (sl % SBW)

    dcore = core_of[col]
    dl = local_of[col]
    dblk = dl // 128
    dloc = dl % 128
    norms = dis[row] * ew * dis[col]

    key = dcore * B + dblk
    # secondary sort by source id: gather descriptors sweep the tables in
    # ascending address order (HBM row-buffer friendly) instead of randomly
    order_e = np.argsort(key * (NP + 1) + srcA, kind="stable")
    key_s = key[order_e]
    counts = np.bincount(key_s, minlength=C * B)
    starts = np.zeros(C * B, dtype=np.int64)
    np.cumsum(counts[:-1], out=starts[1:])
    pos = np.arange(key_s.size, dtype=np.int64) - starts[key_s]

    KT = max(1, int(math.ceil(counts.max() / 128)))

    srcA_s = srcA[order_e].astype(np.int64)
    srcB_s = srcB[order_e].astype(np.int64)
    dcore_s = dcore[order_e]
    dblk_s = dblk[order_e]
    dloc_s = dloc[order_e]
    norm_s = norms[order_e]
    kt_s = pos // 128
    p_s = pos % 128

    # int16 indices for dma_gather: slot i of block b -> [i%16, b*KT*8+i//16],
    # replicated across the 8 groups of 16 partitions
    assert NP < 2 ** 15
    slot = kt_s * 128 + p_s

    def _mk_idx(vals):
        i16 = np.zeros((C, 16, B * KT * 8), dtype=np.int16)
        i16[dcore_s, slot % 16, dblk_s * (KT * 8) + slot // 16] = \
            vals.astype(np.int16)
        return np.tile(i16, (1, 8, 1))

    idxa_all = _mk_idx(srcA_s)
    idxb_all = _mk_idx(srcB_s)

    # S_all[c, p, b, kt*128+d] = norm  (partition-major, contiguous per core)
    S_all = np.zeros((C, 128, B, KT * 128), dtype=np.float32)
    S_all[dcore_s, p_s, dblk_s, kt_s * 128 + dloc_s] = norm_s
    S_all = S_all.astype(np_cdt)

    # ---- node tables (permuted) ----
    x = np.asarray(x, dtype=np.float32)
    x_table = np.zeros((NP, F), dtype=np.float32)
    x_table[core_of * NLP + local_of] = x

    dis2_table = np.zeros(NP, dtype=np.float32)
    dis2_table[core_of * NLP + local_of] = dis * dis

    xd2_table = x_table * dis2_table[:, None]

    fp8 = FP8_GATHER and np_cdt == ml_dtypes.bfloat16
    np_gdt = mybir.dt.np(mybir.dt.float8e4) if fp8 else np_cdt
    x_table_c = x_table.astype(np_gdt)
    # feature-major per core: xT[p, k, n] = x_core[n, k*128+p]
    xT_all = np.ascontiguousarray(
        x_table.astype(np_cdt).reshape(C, NLP, FK, 128).transpose(0, 3, 2, 1))
    xd2T_all = np.ascontiguousarray(
        xd2_table.astype(np_cdt).reshape(C, NLP, FK, 128).transpose(0, 3, 2, 1))
    dis2_in_all = np.ascontiguousarray(
        dis2_table.reshape(C, B, 128).transpose(0, 2, 1))  # [C, 128, B]

    w1 = np.asarray(w1, np.float32)
    rw1 = np.asarray(rw1, np.float32)
    w2 = np.asarray(w2, np.float32)
    rw2 = np.asarray(rw2, np.float32)
    b1c = (np.asarray(b1, np.float32) + np.asarray(rb1, np.float32))
    b2c = (np.asarray(b2, np.float32) + np.asarray(rb2, np.float32))

    # [128, FK, H] : w1_in[p, k, j] = w1[k*128+p, j]
    w1_in = np.ascontiguousarray(
        w1.reshape(FK, 128, H).transpose(1, 0, 2)).astype(np_cdt)
    rw1_in = np.ascontiguousarray(
        rw1.reshape(FK, 128, H).transpose(1, 0, 2)).astype(np_cdt)
    w2_in = np.ascontiguousarray(
        w2.reshape(HC, 128, OUT).transpose(1, 0, 2)).astype(np_cdt)
    rw2b_in = np.zeros((128, HC + 1, OUT), dtype=np.float32)
    rw2b_in[:, :HC] = rw2.reshape(HC, 128, OUT).transpose(1, 0, 2)
    rw2b_in[0, HC, :] = b2c
    rw2b_in = rw2b_in.astype(np_cdt)

    bias1_in = np.ascontiguousarray(b1c.reshape(HC, 128).T).astype(np.float32)

    in_maps = []
    for c in range(C):
        in_maps.append({
            "x_table": x_table_c,
            "idxa_in": np.ascontiguousarray(idxa_all[c]),
            "idxb_in": np.ascontiguousarray(idxb_all[c]),
            "s_in": np.ascontiguousarray(S_all[c]),
            "xt_in": np.ascontiguousarray(xT_all[c]),
            "xd2t_in": np.ascontiguousarray(xd2T_all[c]),
            "dis2_in": dis2_in_all[c],
            "w1_in": w1_in,
            "rw1_in": rw1_in,
            "w2_in": w2_in,
            "rw2b_in": rw2b_in,
            "bias1_in": bias1_in,
        })
    meta = {"core_of": core_of, "local_of": local_of}
    return in_maps, KT, meta


def _assemble(outs_per_core, cfg, meta):
    """outs_per_core[c] is the [NLP, OUT] 'out' tensor of core c; undo the
    node permutation and return the [N, OUT] full output."""
    N, NLP, OUT = cfg["N"], cfg["NLP"], cfg["OUT"]
    stacked = np.stack([np.asarray(o) for o in outs_per_core])  # [C, NLP, OUT]
    full = stacked.reshape(C * NLP, OUT)
    idx = meta["core_of"] * NLP + meta["local_of"]
    return np.ascontiguousarray(full[idx].astype(np.float32))


# ----------------------------------------------------------------------------
# device program
# ----------------------------------------------------------------------------
def _build(cfg, KT, cdt, reps=1, no_collective=False, phases="ab",
           nq=4, scratch=49152, bulk="", pad_dup=False):
    F, H, OUT = cfg["F"], cfg["H"], cfg["OUT"]
    NLP, B, NP = cfg["NLP"], cfg["B"], cfg["NP"]
    FK, HC = cfg["FK"], cfg["HC"]
    SBB, NSB, SBW = cfg["SBB"], cfg["NSB"], cfg["SBW"]
    f32 = mybir.dt.float32
    fp8 = FP8_GATHER and cdt == mybir.dt.bfloat16
    gdt = mybir.dt.float8e4 if fp8 else cdt

    # 32 KB SWDGE scratch = 2048-descriptor ring: a half-block gather
    # (ceil(KT/2)*128 = ~640 descriptors in one instruction) must fit.
    nc = bacc.Bacc("TRN2", target_bir_lowering=False, debug=False,
                   enable_asserts=False, num_devices=C,
                   dynamic_dma_scratch_size=scratch, num_swdge_queues=nq)

    x_table = nc.dram_tensor("x_table", [NP, F], gdt, kind="ExternalInput")
    idxa_in = nc.dram_tensor("idxa_in", [128, B * KT * 8], mybir.dt.int16,
                             kind="ExternalInput")
    idxb_in = nc.dram_tensor("idxb_in", [128, B * KT * 8], mybir.dt.int16,
                             kind="ExternalInput")
    s_in = nc.dram_tensor("s_in", [128, B, KT * 128], cdt,
                          kind="ExternalInput")
    xt_in = nc.dram_tensor("xt_in", [128, FK, NLP], cdt, kind="ExternalInput")
    xd2t_in = nc.dram_tensor("xd2t_in", [128, FK, NLP], cdt,
                             kind="ExternalInput")
    dis2_in = nc.dram_tensor("dis2_in", [128, B], f32, kind="ExternalInput")
    w1_in = nc.dram_tensor("w1_in", [128, FK, H], cdt, kind="ExternalInput")
    rw1_in = nc.dram_tensor("rw1_in", [128, FK, H], cdt, kind="ExternalInput")
    w2_in = nc.dram_tensor("w2_in", [128, HC, OUT], cdt, kind="ExternalInput")
    rw2b_in = nc.dram_tensor("rw2b_in", [128, HC + 1, OUT], cdt,
                             kind="ExternalInput")
    bias1_in = nc.dram_tensor("bias1_in", [128, HC], f32,
                              kind="ExternalInput")
    out_d = nc.dram_tensor("out", [NLP, OUT], f32, kind="ExternalOutput")

    assert 8 % nq == 0, nq
    _qctr = [0]

    def _gather(out_tile, table, idx_sb, b, elem):
        h0 = KT // 2
        ranges = [(0, h0), (h0, KT)] if h0 > 0 else [(0, KT)]
        for lo, hi in ranges:
            nc.gpsimd.dma_gather(
                out_ap=out_tile[:, lo:hi], in_ap=table[:],
                idxs_ap=idx_sb[:, b * KT * 8 + lo * 8:
                               b * KT * 8 + hi * 8],
                num_idxs=(hi - lo) * 128, num_idxs_reg=(hi - lo) * 128,
                elem_size=elem, single_packet=False,
                queue_num=_qctr[0] % nq)
            _qctr[0] += 1

    with tile.TileContext(nc) as tc:
        with (
            tc.tile_pool(name="dram", bufs=1, space="DRAM") as dram,
            tc.tile_pool(name="const", bufs=1) as const,
        ):
            # resident constants
            w1_sb = const.tile([128, FK, H], cdt)
            nc.sync.dma_start(out=w1_sb[:], in_=w1_in[:])
            rw1_sb = const.tile([128, FK, H], cdt)
            nc.sync.dma_start(out=rw1_sb[:], in_=rw1_in[:])
            w2_sb = const.tile([128, HC, OUT], cdt)
            nc.sync.dma_start(out=w2_sb[:], in_=w2_in[:])
            rw2b_sb = const.tile([128, HC + 1, OUT], cdt)
            nc.sync.dma_start(out=rw2b_sb[:], in_=rw2b_in[:])
            bias1_sb = const.tile([128, HC], f32)
            nc.sync.dma_start(out=bias1_sb[:], in_=bias1_in[:])
            dis2_sb = const.tile([128, B], f32)
            nc.sync.dma_start(out=dis2_sb[:], in_=dis2_in[:])
            idxa_sb = const.tile([128, B * KT * 8], mybir.dt.int16)
            nc.sync.dma_start(out=idxa_sb[:], in_=idxa_in[:])
            idxb_sb = const.tile([128, B * KT * 8], mybir.dt.int16)
            nc.sync.dma_start(out=idxb_sb[:], in_=idxb_in[:])
            ones_sb = const.tile([128, 128], cdt)
            nc.vector.memset(ones_sb[:], 0.0)
            nc.vector.memset(ones_sb[0:1, :], 1.0)

            # resident: S (norm selection matrices)
            s_all_sb = const.tile([128, B, KT * 128], cdt)
            nc.sync.dma_start(out=s_all_sb[:], in_=s_in[:])

            if "a" in bulk:
                bulk_a = dram.tile([128, KT, F], gdt, tag="bulk_a",
                                   name="bulk_a")
            if "b" in bulk:
                bulk_b = dram.tile([128, KT, 2 * OUT if fp8 else OUT],
                                   gdt if fp8 else cdt, tag="bulk_b",
                                   name="bulk_b")

            # shared pools across reps: consecutive passes software-pipeline
            # (rep k's phase B weaves into rep k+1's phase A emission), so
            # PE/SDMA stay busy through the AllGather chain.
            with (
                tc.tile_pool(name="xg_pool", bufs=2) as xg_pool,
                tc.tile_pool(name="xt_pool", bufs=1) as xt_pool,
                tc.tile_pool(name="ax_pool", bufs=2) as ax_pool,
                tc.tile_pool(name="ht_pool", bufs=2) as ht_pool,
                tc.tile_pool(name="osb8_pool", bufs=3) as osb8_pool,
                tc.tile_pool(name="hwg_pool", bufs=4) as hwg_pool,
                tc.tile_pool(name="resg_pool", bufs=3) as resg_pool,
                tc.tile_pool(name="osb_pool", bufs=3) as osb_pool,
                tc.tile_pool(name="ax_psum", bufs=2, space="PSUM") as ax_psum,
                tc.tile_pool(name="h_psum", bufs=2, space="PSUM") as h_psum,
                tc.tile_pool(name="hwr_psum", bufs=2,
                             space="PSUM") as hwr_psum,
                tc.tile_pool(name="o_psum", bufs=2, space="PSUM") as o_psum,
            ):
                def phase_a_sb(rep, sb, hw_loc, hw_full):
                    ss = slice(sb * SBW, (sb + 1) * SBW)
                    xt4 = xt_pool.tile([128, FK, SBW], cdt, tag="xt4")
                    nc.sync.dma_start(out=xt4[:], in_=xt_in[:, :, ss])
                    xd2t4 = xt_pool.tile([128, FK, SBW], cdt, tag="xd2t4")
                    nc.sync.dma_start(out=xd2t4[:], in_=xd2t_in[:, :, ss])
                    axT4 = ax_pool.tile([128, FK, SBW], cdt, tag="axT4")

                    for j in range(SBB):
                        b = sb * SBB + j
                        xg8 = xg_pool.tile([128, KT, F], gdt, tag="xg8")
                        if "a" in bulk:
                            nc.sync.dma_start(out=xg8[:], in_=bulk_a[:])
                        else:
                            _gather(xg8, x_table, idxa_sb, b, F)
                        if fp8:
                            xg = xg_pool.tile([128, KT, F], cdt, tag="xg")
                            h0 = max(KT // 2, 1)
                            nc.vector.tensor_copy(out=xg[:, 0:h0],
                                                  in_=xg8[:, 0:h0])
                            if h0 < KT:
                                nc.vector.tensor_copy(out=xg[:, h0:],
                                                      in_=xg8[:, h0:])
                        else:
                            xg = xg8
                        psum_ax = ax_psum.tile([128, FK, 128], f32,
                                               tag="psum_ax")
                        for fc in range(FK):
                            for kt in range(KT):
                                nc.tensor.matmul(
                                    out=psum_ax[:, fc, :],
                                    lhsT=xg[:, kt, fc * 128:(fc + 1) * 128],
                                    rhs=s_all_sb[:, b,
                                                 kt * 128:(kt + 1) * 128],
                                    start=(kt == 0), stop=(kt == KT - 1))
                        # PSUM evac + self-loop add (dis^2 * x), cast bf16
                        nc.vector.tensor_tensor(
                            out=axT4[:, :, j * 128:(j + 1) * 128],
                            in0=psum_ax[:],
                            in1=xd2t4[:, :, j * 128:(j + 1) * 128],
                            op=mybir.AluOpType.add)

                    # dense: hT = relu(W1.T @ axT + RW1.T @ xT + b1c),
                    # free dim SBW
                    hT4 = ht_pool.tile([128, HC, SBW], cdt, tag="hT4")
                    for hc in range(HC):
                        hs = slice(hc * 128, (hc + 1) * 128)
                        psum_h = h_psum.tile([128, SBW], f32, tag="psum_h")
                        for k in range(FK):
                            nc.tensor.matmul(
                                out=psum_h[:], lhsT=w1_sb[:, k, hs],
                                rhs=axT4[:, k, :],
                                start=(k == 0), stop=False)
                        for k in range(FK):
                            nc.tensor.matmul(
                                out=psum_h[:], lhsT=rw1_sb[:, k, hs],
                                rhs=xt4[:, k, :],
                                start=False, stop=(k == FK - 1))
                        nc.scalar.activation(
                            out=hT4[:, hc, :], in_=psum_h[:],
                            func=mybir.ActivationFunctionType.Relu,
                            bias=bias1_sb[:, hc:hc + 1], scale=1.0)

                    # per block: hw = h @ W2 (wire, fp8) and the local part
                    # of the output: res = h @ RW2 + b2c + dis^2 * hw
                    for j in range(SBB):
                        b = sb * SBB + j
                        bw = slice(b * 128, (b + 1) * 128)
                        js = slice(j * 128, (j + 1) * 128)
                        psum_wr = hwr_psum.tile([128, 2, OUT], f32,
                                                tag="psum_wr")
                        for hc in range(HC):
                            nc.tensor.matmul(
                                out=psum_wr[:, 0, :], lhsT=hT4[:, hc, js],
                                rhs=w2_sb[:, hc, :],
                                start=(hc == 0), stop=(hc == HC - 1))
                        for hc in range(HC):
                            nc.tensor.matmul(
                                out=psum_wr[:, 1, :], lhsT=hT4[:, hc, js],
                                rhs=rw2b_sb[:, hc, 0:OUT],
                                start=(hc == 0), stop=False)
                        nc.tensor.matmul(
                            out=psum_wr[:, 1, :], lhsT=ones_sb[:],
                            rhs=rw2b_sb[:, HC, 0:OUT],
                            start=False, stop=True)
                        hw8 = osb8_pool.tile([128, OUT], gdt, tag="hw8")
                        nc.vector.tensor_copy(out=hw8[:], in_=psum_wr[:, 0, :])
                        nc.sync.dma_start(out=hw_loc[bw, :], in_=hw8[:])
                        tmp = osb8_pool.tile([128, OUT], f32, tag="tmp")
                        nc.vector.tensor_scalar(
                            out=tmp[:], in0=psum_wr[:, 0, :],
                            scalar1=dis2_sb[:, b:b + 1], scalar2=None,
                            op0=mybir.AluOpType.mult)
                        res_sb = osb8_pool.tile([128, OUT], cdt, tag="res_sb")
                        nc.vector.tensor_tensor(
                            out=res_sb[:], in0=psum_wr[:, 1, :], in1=tmp[:],
                            op=mybir.AluOpType.add)
                        nc.sync.dma_start(out=res_d[rep][bw, :],
                                          in_=res_sb[:])

                    # all-gather this superblock's hw chunk
                    if no_collective:
                        nc.sync.dma_start(
                            out=hw_full[sb * C * SBW:sb * C * SBW + SBW,
                                        0:OUT],
                            in_=hw_loc[ss, :])
                        if fp8 and pad_dup:
                            nc.sync.dma_start(
                                out=hw_full[sb * C * SBW:sb * C * SBW + SBW,
                                            OUT:2 * OUT],
                                in_=hw_loc[ss, :])
                    else:
                        hw_ag = dram.tile([C * SBW, OUT], gdt,
                                          addr_space="Shared",
                                          tag=f"hw_ag{rep}_{sb}",
                                          name=f"hw_ag{rep}_{sb}")
                        nc.gpsimd.collective_compute(
                            "AllGather",
                            mybir.AluOpType.bypass,
                            replica_groups=[list(range(C))],
                            ins=[hw_loc[ss, :].opt()],
                            outs=[hw_ag[:].opt()],
                        )
                        nc.sync.dma_start(
                            out=hw_full[sb * C * SBW:
                                        (sb + 1) * C * SBW, 0:OUT],
                            in_=hw_ag[:])
                        if fp8 and pad_dup:
                            # sim-only: keep the never-read pad half finite
                            nc.sync.dma_start(
                                out=hw_full[sb * C * SBW:
                                            (sb + 1) * C * SBW, OUT:2 * OUT],
                                in_=hw_ag[:])

                def phase_b_blocks(rep, blocks, hw_full):
                    for b in blocks:
                        bw = slice(b * 128, (b + 1) * 128)
                        if fp8:
                            hwg8 = hwg_pool.tile([128, KT, 2 * OUT], gdt,
                                                 tag="hwg8")
                            if "b" in bulk:
                                nc.sync.dma_start(out=hwg8[:], in_=bulk_b[:])
                            else:
                                _gather(hwg8, hw_full, idxb_sb, b, 2 * OUT)
                            hwg = hwg_pool.tile([128, KT, OUT], cdt,
                                                tag="hwg")
                            nc.vector.tensor_copy(out=hwg[:],
                                                  in_=hwg8[:, :, 0:OUT])
                        else:
                            hwg = hwg_pool.tile([128, KT, OUT], cdt,
                                                tag="hwg")
                            if "b" in bulk:
                                nc.sync.dma_start(out=hwg[:], in_=bulk_b[:])
                            else:
                                _gather(hwg, hw_full, idxb_sb, b, OUT)
                        resg = resg_pool.tile([128, OUT], cdt, tag="resg")
                        nc.sync.dma_start(out=resg[:], in_=res_d[rep][bw, :])
                        psum_o = o_psum.tile([128, OUT], f32, tag="psum_o")
                        for kt in range(KT):
                            nc.tensor.matmul(
                                out=psum_o[:],
                                lhsT=s_all_sb[:, b, kt * 128:(kt + 1) * 128],
                                rhs=hwg[:, kt, :],
                                start=(kt == 0), stop=(kt == KT - 1))
                        out_sb = osb_pool.tile([128, OUT], f32, tag="out_sb")
                        nc.vector.tensor_tensor(
                            out=out_sb[:], in0=psum_o[:], in1=resg[:],
                            op=mybir.AluOpType.add)
                        nc.sync.dma_start(out=out_d[bw, :], in_=out_sb[:])

                hw_fulls, res_d = [], []
                gw = 2 * OUT if fp8 else OUT   # padded row width (fp8)
                for rep in range(reps):
                    hw_fulls.append(dram.tile(
                        [NSB * C * SBW, gw], gdt if fp8 else cdt,
                        tag=f"hw_full{rep}", name=f"hw_full{rep}"))
                    res_d.append(dram.tile(
                        [NLP, OUT], cdt, tag=f"res_d{rep}",
                        name=f"res_d{rep}"))

                for rep in range(reps):
                    hw_loc = dram.tile([NLP, OUT], gdt, tag=f"hw_loc{rep}",
                                       name=f"hw_loc{rep}")
                    for sb in range(NSB):
                        phase_a_sb(rep, sb, hw_loc, hw_fulls[rep])
                        if rep > 0 and phases != "a":
                            phase_b_blocks(
                                rep - 1,
                                range(sb * SBB, (sb + 1) * SBB),
                                hw_fulls[rep - 1])
                if phases != "a":
                    phase_b_blocks(reps - 1, range(B), hw_fulls[reps - 1])

    nc.compile()
    return nc


# ----------------------------------------------------------------------------
# entry points
# ----------------------------------------------------------------------------
def _run(inputs, trace=False, compute=None, trace_kwargs=None):
    compute = compute or COMPUTE
    cdt, np_cdt = _DT[compute]
    x = np.asarray(inputs["x"])
    cfg = _cfg_from_shapes(x, np.asarray(inputs["w1"]),
                           np.asarray(inputs["w2"]))
    in_maps, KT, meta = _preprocess(
        x, inputs["edge_index"], inputs["edge_weight"],
        inputs["w1"], inputs["b1"], inputs["w2"], inputs["b2"],
        inputs["rw1"], inputs["rb1"], inputs["rw2"], inputs["rb2"],
        cfg, np_cdt)

    key = (tuple(sorted(cfg.items())), KT, compute)
    nc = _prog_cache.get(key)
    if nc is None:
        nc = _build(cfg, KT, cdt)
        _prog_cache[key] = nc

    res = run_bass_kernel_spmd(
        nc, in_maps, core_ids=list(range(C)), trace=trace,
        **(trace_kwargs or {}))

    out = _assemble([res.results[c]["out"] for c in range(C)], cfg, meta)
    return out, res


def kernel(**inputs) -> np.ndarray:
    out, _ = _run(inputs, trace=False)
    return out
